# revision 22
# baseline (speedup 1.0000x reference)
"""CvT attention block on 8 trn2 NeuronCores — batch-parallel (1 image/core).

Pipeline per core (channel-major activations [C_part, tok_free]):
  DMA token-major x -> PE-transpose -> zero-padded [C,58*58] plane
  -> depthwise 3x3 = 9 PSUM-accumulated diag-weight matmuls (BN scale folded)
  -> pointwise conv matmuls (BN-shift bias folded into pw bias, added on DVE)
  -> per 448-token q-tile: QK^T (scores transposed, kv on partitions),
     exp on ACT, AV with ones-column for the softmax denominator,
     reciprocal + gpsimd ones-broadcast normalize, bf16 out-projection into
     token-major PSUM -> DMA.
  Schedule: x DMAs issue ahead of weight DMAs; dw-q matmuls of tile t+1 are
  interleaved between attention channel groups of tile t (PE fill work while
  the scalar engine drains exp); pw-q of t+1 issues before op(t) so qt lands
  early; on the vector queue, den/rec and the dw-q evacuation precede the
  gpsimd-broadcast-gated normalize mults so PSUM banks free without
  cross-engine stalls.
"""
import sys

if '/opt/trn_rl_repo' not in sys.path:
    sys.path.insert(0, '/opt/trn_rl_repo')

from contextlib import ExitStack

import numpy as np

import concourse.bass as bass
import concourse.tile as tile
from concourse import mybir, bacc
from concourse.bass_utils import run_bass_kernel_spmd

F32 = mybir.dt.float32
BF16 = mybir.dt.bfloat16
AF = mybir.ActivationFunctionType
MUL = mybir.AluOpType.mult
ADD = mybir.AluOpType.add

B, H, W, C = 8, 56, 56, 384
NH, HD = 6, 64
NTOK = H * W            # 3136
NKV = 28 * 28           # 784
PW = 58                 # padded plane width
QT = 448                # q token tile = 8 image rows
NQT = NTOK // QT        # 7
KVC = 112               # kv chunk (attention contraction tile)
NKVC = NKV // KVC       # 7
BN_EPS = 1e-5

_cache = {}


def _build_nc():
    nc = bacc.Bacc("TRN2", target_bir_lowering=False, debug=False)
    d = {}
    ext = lambda n, s: nc.dram_tensor(n, s, F32, kind="ExternalInput").ap()
    d['xq'] = ext("xq", [NTOK, C])
    d['xkv'] = ext("xkv", [NTOK, C])
    for p in 'qkv':
        d[f'w{p}'] = ext(f"w{p}", [C, C])          # pw weights (cin, cout)
        d[f'dwv{p}'] = ext(f"dwv{p}", [128, 27])   # diag values (p, tap*3+ch)
    d['bq'] = ext("bq", [128, 3])
    d['bk'] = ext("bk", [128, 3])
    d['bv'] = ext("bv", [1, C])
    d['ok'] = ext("ok", [C, C])                    # out_kernel (hd, o)
    d['mask'] = ext("mask", [128, 128])            # identity mask for diag build
    d['ident'] = ext("ident", [112, 112])          # transpose identity
    out_d = nc.dram_tensor("out", [NTOK, C], F32, kind="ExternalOutput").ap()

    with nc.allow_low_precision(reason="bf16/fp8 rounding is intentional"), \
         tile.TileContext(nc) as tc, ExitStack() as ctx:
        wp = ctx.enter_context(tc.tile_pool(name="wp", bufs=1))
        stg = ctx.enter_context(tc.tile_pool(name="stg", bufs=2))
        xp = ctx.enter_context(tc.tile_pool(name="xp", bufs=3))
        padp = ctx.enter_context(tc.tile_pool(name="padp", bufs=1))
        dgp = ctx.enter_context(tc.tile_pool(name="dgp", bufs=1))
        dwo = ctx.enter_context(tc.tile_pool(name="dwo", bufs=1))
        dwq = ctx.enter_context(tc.tile_pool(name="dwq", bufs=2))
        actp = ctx.enter_context(tc.tile_pool(name="actp", bufs=1))
        qtp = ctx.enter_context(tc.tile_pool(name="qtp", bufs=2))
        ptp = ctx.enter_context(tc.tile_pool(name="ptp", bufs=1))
        smp = ctx.enter_context(tc.tile_pool(name="smp", bufs=2))
        ap_ = ctx.enter_context(tc.tile_pool(name="ap", bufs=2))
        outp = ctx.enter_context(tc.tile_pool(name="outp", bufs=2))
        psu = ctx.enter_context(tc.tile_pool(name="psu", bufs=1, space="PSUM"))
        _n = [0]

        def nm(s):
            _n[0] += 1
            return f"{s}_{_n[0]}"

        def load_f32(name, shape, tag):
            t = wp.tile(shape, F32, tag=tag, name=nm(tag))
            nc.sync.dma_start(t[:], d[name][:, :])
            return t

        def load_cast(name, shape, tag, dt=BF16, rows=None):
            s = stg.tile(shape, F32, tag="stg", name=nm("stg"))
            src = d[name][:, :] if rows is None else d[name][rows[0]:rows[1], :]
            nc.sync.dma_start(s[:], src)
            t = wp.tile(shape, dt, tag=tag, name=nm(tag))
            nc.vector.tensor_copy(t[:], s[:])
            return t

        ident = load_f32('ident', [112, 112], "ident")
        identb = wp.tile([112, 112], BF16, tag="identb", name=nm("identb"))
        nc.vector.tensor_copy(identb[:], ident[:])

        ones1 = wp.tile([1, KVC], F32, tag="ones1", name=nm("ones1"))
        nc.vector.memset(ones1[:], 1.0)
        ones1r = wp.tile([1, KVC], BF16, tag="ones1r", name=nm("ones1r"))
        nc.vector.tensor_copy(ones1r[:], ones1[:])
        onesv = wp.tile([112, NH], F32, tag="onesv", name=nm("onesv"))
        nc.vector.memset(onesv[:], 1.0)
        zrow = wp.tile([128, PW], F32, tag="zrow", name=nm("zrow"))
        nc.vector.memset(zrow[:], 0.0)

        def zero_border(pad):
            pv = pad[:].rearrange("p (r c) -> p r c", c=PW)
            zr = zrow[:].rearrange("p (a c) -> p a c", a=1)
            zc = zrow[:].rearrange("p (c a) -> p c a", a=1)
            nc.vector.tensor_copy(pv[:, 0:1, :], zr)
            nc.vector.tensor_copy(pv[:, PW - 1:PW, :], zr)
            nc.vector.tensor_copy(pv[:, :, 0:1], zc)
            nc.vector.tensor_copy(pv[:, :, PW - 1:PW], zc)

        def transpose_in(src_dram, pads, scatter_engines):
            """DMA token-major rows, cast bf16, PE-transpose, scatter into pads."""
            for g in range(28):  # 112 tokens = 2 image rows per group
                xs = xp.tile([112, C], F32, tag="xs", name=nm("xs"))
                nc.sync.dma_start(xs[:], src_dram[g * 112:(g + 1) * 112, :])
                xb = xp.tile([112, C], BF16, tag="xb", name=nm("xb"))
                if g % 2 == 0:
                    nc.vector.tensor_copy(xb[:], xs[:])
                else:
                    nc.scalar.copy(xb[:], xs[:])
                for ch in range(3):
                    pt = psu.tile([128, 112], BF16, tag="qk", name=nm("qk"), bufs=2)
                    nc.tensor.transpose(pt[:], xb[:, ch * 128:(ch + 1) * 128], identb[:])
                    pv = pads[ch][:].rearrange("p (r c) -> p r c", c=PW)
                    dst = pv[:, 2 * g + 1:2 * g + 3, 1:57]
                    src = pt[:].rearrange("p (r c) -> p r c", c=56)
                    eng = scatter_engines[(g + ch) % len(scatter_engines)]
                    if eng == 'v':
                        nc.vector.tensor_copy(dst, src)
                    else:
                        nc.scalar.copy(dst, src)

        def build_diags(p):
            tg = 'q' if p == 'q' else 'kv'
            dgs = []
            for ch in range(3):
                row = []
                for tap in range(9):
                    dg = dgp.tile([128, 128], BF16, tag=f"dg{tg}{ch}_{tap}",
                                  name=nm(f"dg{p}{ch}_{tap}"))
                    nc.vector.tensor_scalar(
                        dg[:], mask[:], dwv[p][:, tap * 3 + ch:tap * 3 + ch + 1],
                        None, op0=MUL)
                    row.append(dg)
                dgs.append(row)
            return dgs

        # ---- KV input feed first: x DMAs ahead of weight DMAs ----
        pads_kv = [padp.tile([128, PW * PW], BF16, tag=f"pad{ch}", name=nm(f"pad{ch}")) for ch in range(3)]
        for ch in range(3):
            zero_border(pads_kv[ch])
        transpose_in(d['xkv'], pads_kv, ['v', 's'])

        mask = load_f32('mask', [128, 128], "mask")
        dwv = {p: load_f32(f'dwv{p}', [128, 27], f"dwv{p}") for p in 'qkv'}
        bq = load_f32('bq', [128, 3], "bq")
        bk = load_f32('bk', [128, 3], "bk")
        bv = load_cast('bv', [1, C], "bv")
        wmat = {p: [load_cast(f'w{p}', [128, C], f"w{p}{c}", rows=(c * 128, (c + 1) * 128))
                    for c in range(3)] for p in 'qkv'}
        okm = [load_cast('ok', [128, C], f"ok{c}", rows=(c * 128, (c + 1) * 128))
               for c in range(3)]

        pads_q = [padp.tile([128, PW * PW], BF16, tag=f"padq{ch}", name=nm(f"padq{ch}")) for ch in range(3)]
        for ch in range(3):
            zero_border(pads_q[ch])
        transpose_in(d['xq'], pads_q, ['v', 's'])

        kvdw = {}
        for p in 'kv':
            dgs = build_diags(p)
            for ch in range(3):
                ot = dwo.tile([128, NKV], BF16, tag=f"kvdw_{p}{ch}", name=nm(f"kvdw_{p}{ch}"))
                kvdw[(p, ch)] = ot
                for half in range(2):  # 392 tokens = 14 out rows of 28
                    ps = psu.tile([128, 392], F32, tag="pb", name=nm("pb"), bufs=2)
                    pv = pads_kv[ch][:].rearrange("p (r c) -> p r c", c=PW)
                    for tap in range(9):
                        dy, dx = tap // 3, tap % 3
                        y0 = half * 14
                        rv = pv[:, 2 * y0 + dy + 1: 2 * y0 + dy + 28:2,
                                dx + 1: dx + 56:2]
                        nc.tensor.matmul(ps[:], dgs[ch][tap][:], rv,
                                         start=(tap == 0), stop=(tap == 8))
                    dst = ot[:, half * 392:(half + 1) * 392]
                    if (ch + half) % 2 == 0:
                        nc.vector.tensor_copy(dst, ps[:])
                    else:
                        nc.scalar.copy(dst, ps[:])

        # pw-k: channel-major kT [3][128, 784], bias added on DVE
        kT = []
        for co in range(3):
            kt = actp.tile([128, NKV], BF16, tag=f"kT{co}", name=nm(f"kT{co}"))
            kT.append(kt)
            for half in range(2):
                ps = psu.tile([128, 392], F32, tag="pb", name=nm("pb"), bufs=2)
                for ci in range(3):
                    nc.tensor.matmul(
                        ps[:], wmat['k'][ci][:, co * 128:(co + 1) * 128],
                        kvdw[('k', ci)][:, half * 392:(half + 1) * 392],
                        start=(ci == 0), stop=(ci == 2))
                nc.vector.tensor_scalar(kt[:, half * 392:(half + 1) * 392], ps[:],
                                        bk[:, co:co + 1], None, op0=ADD)

        # pw-v: token-major v' [7][112, 6*65] with ones col per head
        vs = []
        for j in range(NKVC):
            ps = psu.tile([112, C], F32, tag="pb", name=nm("pb"), bufs=2)
            for ci in range(3):
                nc.tensor.matmul(ps[:], kvdw[('v', ci)][:, j * KVC:(j + 1) * KVC],
                                 wmat['v'][ci][:], start=(ci == 0), stop=False)
            nc.tensor.matmul(ps[:], ones1r[:], bv[:], start=False, stop=True)
            vt = actp.tile([112, NH * 65], BF16, tag=f"vs{j}", name=nm(f"vs{j}"))
            vs.append(vt)
            vv = vt[:].rearrange("p (h e) -> p h e", e=65)
            nc.scalar.copy(vv[:, :, 0:64],
                           ps[:].rearrange("p (h e) -> p h e", e=64))
            nc.scalar.copy(vv[:, :, 64:65],
                           onesv[:].rearrange("p (h e) -> p h e", e=1))

        # ---- Q path + attention + projection, per 448-token tile ----
        # Software-pipelined: dw-q matmuls of tile t+1 are interleaved between
        # the attention channel groups of tile t so the PE has fill work while
        # the scalar engine drains the exp queue; pw-q of t+1 follows op(t).
        dgs_q = build_diags('q')

        def do_dwq_ch(t, ch):
            y0 = t * 8
            ps = psu.tile([128, QT], F32, tag="pb", name=nm("pb"), bufs=2)
            pv = pads_q[ch][:].rearrange("p (r c) -> p r c", c=PW)
            for tap in range(9):
                dy, dx = tap // 3, tap % 3
                rv = pv[:, y0 + dy:y0 + dy + 8, dx:dx + 56]
                nc.tensor.matmul(ps[:], dgs_q[ch][tap][:], rv,
                                 start=(tap == 0), stop=(tap == 8))
            dt_ = dwq.tile([128, QT], BF16, tag=f"dwq{ch}", name=nm(f"dwq{ch}"), bufs=3)
            nc.vector.tensor_copy(dt_[:], ps[:])
            return dt_

        def do_pwq(dq):
            qt_ = []
            for co in range(3):
                ps = psu.tile([128, QT], F32, tag="pb", name=nm("pb"), bufs=2)
                for ci in range(3):
                    nc.tensor.matmul(ps[:], wmat['q'][ci][:, co * 128:(co + 1) * 128],
                                     dq[ci][:], start=(ci == 0), stop=(ci == 2))
                qtt = qtp.tile([128, QT], BF16, tag=f"qt{co}", name=nm(f"qt{co}"), bufs=7)
                qt_.append(qtt)
                nc.scalar.activation(qtt[:], ps[:], AF.Identity, bias=bq[:, co:co + 1])
            return qt_

        qt_ = do_pwq([do_dwq_ch(0, ch) for ch in range(3)])

        for t in range(NQT):
            dq_next = [None, None, None]
            # attention: heads processed in base-partition pairs; QK row-packed
            at_ = [ap_.tile([128, QT], BF16, tag=f"at{ch}", name=nm(f"at{ch}")) for ch in range(3)]
            for ch in range(3):
                hpts = {0: [], 1: []}
                for j2 in range(3):
                    pss = {}
                    for half in range(2):
                        lo = half * 64
                        ps = psu.tile([112, 1024], F32, tag="qk", name=nm("qk"), bufs=2)
                        pss[half] = ps
                        for k in range(2):
                            j = 2 * j2 + k
                            nc.tensor.matmul(ps[:, k * 512:k * 512 + QT],
                                             kT[ch][lo:lo + 64, j * KVC:(j + 1) * KVC],
                                             qt_[ch][lo:lo + 64, :], start=True, stop=True,
                                             skip_group_check=True)
                    for half in range(2):
                        pt = ptp.tile([112, 2 * QT], BF16, tag=f"pt{half}_{j2}",
                                      name=nm(f"pt{half}_{j2}"), bufs=2)
                        hpts[half].append(pt)
                        nc.scalar.activation(
                            pt[:].rearrange("p (a q) -> p a q", a=2),
                            pss[half][:].rearrange("p (a q) -> p a q", a=2)[:, :, 0:QT],
                            AF.Exp)
                pt6s = {}
                for half in range(2):
                    lo = half * 64
                    ps = psu.tile([112, 1024], F32, tag="qk", name=nm("qk"), bufs=2)
                    nc.tensor.matmul(ps[:, 0:QT], kT[ch][lo:lo + 64, 6 * KVC:7 * KVC],
                                     qt_[ch][lo:lo + 64, :], start=True, stop=True)
                    pt6 = ptp.tile([112, QT], BF16, tag=f"pt{half}_3",
                                   name=nm(f"pt{half}_3"), bufs=2)
                    pt6s[half] = pt6
                    nc.scalar.activation(pt6[:], ps[:, 0:QT], AF.Exp)
                avs, bcs = {}, {}
                for half in range(2):
                    h = 2 * ch + half
                    pts = hpts[half]
                    pt6 = pt6s[half]
                    av = psu.tile([65, QT], F32, tag="mp", name=nm("mp"), bufs=2)
                    avs[half] = av
                    for j in range(NKVC):
                        rhs = pt6[:] if j == 6 else pts[j // 2][:, (j % 2) * QT:(j % 2) * QT + QT]
                        nc.tensor.matmul(av[:], vs[j][:, h * 65:(h + 1) * 65], rhs,
                                         start=(j == 0), stop=(j == NKVC - 1))
                    den = smp.tile([1, QT], F32, tag="den", name=nm("den"))
                    nc.vector.tensor_copy(den[:], av[64:65, :])
                    rec = smp.tile([1, QT], F32, tag="rec", name=nm("rec"))
                    nc.vector.reciprocal_approx_fast(rec[:], den[:])
                    bc = smp.tile([64, QT], F32, tag="bc", name=nm("bc"), bufs=2)
                    bcs[half] = bc
                    nc.gpsimd.partition_broadcast(bc[:], rec[:])
                # dw-q of the next tile: PE fill work during exp/broadcast; its
                # vector evac is issued before the broadcast-gated TT mults so
                # the pb PSUM bank frees without waiting on gpsimd.
                if t + 1 < NQT:
                    dq_next[ch] = do_dwq_ch(t + 1, ch)
                for half in range(2):
                    nc.vector.tensor_tensor(at_[ch][half * 64:half * 64 + 64, :],
                                            avs[half][0:64, :], bcs[half][:],
                                            op=MUL)

            if t + 1 < NQT:
                qt_ = do_pwq(dq_next)
            # out projection, token-major (bf16 stationary + moving)
            for i, qn in ((0, 128), (1, 128), (2, 128), (3, 64)):
                ps = psu.tile([qn, C], F32, tag="mp", name=nm("mp"), bufs=2)
                for ch in range(3):
                    nc.tensor.matmul(ps[:], at_[ch][:, i * 128:i * 128 + qn],
                                     okm[ch][:], start=(ch == 0), stop=(ch == 2))
                ot = outp.tile([qn, C], F32, tag="ot", name=nm("ot"))
                nc.vector.tensor_copy(ot[:], ps[:])
                nc.sync.dma_start(out_d[t * QT + i * 128:t * QT + i * 128 + qn, :],
                                  ot[:])

    nc.compile()
    return nc


def _fold_weights(inputs):
    g = lambda n: np.asarray(inputs[n], dtype=np.float32)
    fold = {}
    for p in 'qkv':
        s = g(f'{p}_bn_scale') / np.sqrt(g(f'{p}_bn_var') + BN_EPS)
        t = g(f'{p}_bn_bias') - g(f'{p}_bn_mean') * s
        dw = g(f'{p}_dw_kernel').reshape(9, C) * s[None, :]         # (tap, c)
        wmat = g(f'{p}_pw_kernel').reshape(C, C)
        bias = t @ wmat
        if p == 'q':
            wmat = wmat / np.sqrt(np.float32(HD))
            bias = bias / np.sqrt(np.float32(HD))
        # dwv[part, tap*3+ch] = dw[tap, ch*128+part]
        dwv = dw.reshape(9, 3, 128).transpose(2, 0, 1).reshape(128, 27)
        fold[f'w{p}'] = np.ascontiguousarray(wmat)
        fold[f'dwv{p}'] = np.ascontiguousarray(dwv)
        fold[f'b{p}'] = bias
    common = {
        'wq': fold['wq'], 'wk': fold['wk'], 'wv': fold['wv'],
        'dwvq': fold['dwvq'], 'dwvk': fold['dwvk'], 'dwvv': fold['dwvv'],
        'bq': np.ascontiguousarray(fold['bq'].reshape(3, 128).T),
        'bk': np.ascontiguousarray(fold['bk'].reshape(3, 128).T),
        'bv': fold['bv'].reshape(1, C),
        'ok': np.ascontiguousarray(np.asarray(inputs['out_kernel'],
                                              dtype=np.float32).reshape(C, C)),
        'mask': np.eye(128, dtype=np.float32),
        'ident': np.eye(112, dtype=np.float32),
    }
    return common


def kernel(**inputs):
    if 'nc' not in _cache:
        _cache['nc'] = _build_nc()
    nc = _cache['nc']
    common = _fold_weights(inputs)
    xq = np.asarray(inputs['inputs_q'], dtype=np.float32).reshape(B, NTOK, C)
    xkv = np.asarray(inputs['inputs_kv'], dtype=np.float32).reshape(B, NTOK, C)
    in_maps = [dict(common, xq=np.ascontiguousarray(xq[b]),
                    xkv=np.ascontiguousarray(xkv[b])) for b in range(B)]
    res = run_bass_kernel_spmd(nc, in_maps, list(range(B)), trace=False)
    out = np.stack([res.results[b]['out'] for b in range(B)], axis=0)
    return out.astype(np.float32)


# revision 23
# speedup vs baseline: 1.1452x; 1.1452x over previous
"""CvT attention block on 8 trn2 NeuronCores — batch-parallel (1 image/core).

Pipeline per core (channel-major activations [C_part, tok_free]):
  DMA token-major x -> PE-transpose -> zero-padded [C,58*58] plane
  -> depthwise 3x3 = 9 PSUM-accumulated diag-weight matmuls (BN scale folded)
  -> pointwise conv matmuls (BN-shift bias folded into pw bias, added on DVE)
  -> per 448-token q-tile: QK^T (scores transposed, kv on partitions),
     exp on ACT, AV with ones-column for the softmax denominator,
     reciprocal + gpsimd ones-broadcast normalize, bf16 out-projection into
     token-major PSUM -> DMA.
  Schedule: x DMAs issue ahead of weight DMAs; dw-q matmuls of tile t+1 are
  interleaved between attention channel groups of tile t (PE fill work while
  the scalar engine drains exp); pw-q of t+1 issues before op(t) so qt lands
  early; on the vector queue, den/rec and the dw-q evacuation precede the
  gpsimd-broadcast-gated normalize mults so PSUM banks free without
  cross-engine stalls.
"""
import sys

if '/opt/trn_rl_repo' not in sys.path:
    sys.path.insert(0, '/opt/trn_rl_repo')

from contextlib import ExitStack

import numpy as np

import concourse.bass as bass
import concourse.tile as tile
from concourse import mybir, bacc
from concourse.bass_utils import run_bass_kernel_spmd

F32 = mybir.dt.float32
BF16 = mybir.dt.bfloat16
AF = mybir.ActivationFunctionType
MUL = mybir.AluOpType.mult
ADD = mybir.AluOpType.add

B, H, W, C = 8, 56, 56, 384
NH, HD = 6, 64
NTOK = H * W            # 3136
NKV = 28 * 28           # 784
PW = 58                 # padded plane width
QT = 448                # q token tile = 8 image rows
NQT = NTOK // QT        # 7
KVC = 112               # kv chunk (attention contraction tile)
NKVC = NKV // KVC       # 7
BN_EPS = 1e-5

_cache = {}


def _build_nc():
    nc = bacc.Bacc("TRN2", target_bir_lowering=False, debug=False)
    d = {}
    ext = lambda n, s: nc.dram_tensor(n, s, F32, kind="ExternalInput").ap()
    d['xq'] = ext("xq", [NTOK, C])
    d['xkv'] = ext("xkv", [NTOK, C])
    for p in 'qkv':
        d[f'w{p}'] = ext(f"w{p}", [C, C])          # pw weights (cin, cout)
        d[f'dwv{p}'] = ext(f"dwv{p}", [128, 27])   # diag values (p, tap*3+ch)
    d['bq'] = ext("bq", [128, 3])
    d['bk'] = ext("bk", [128, 3])
    d['bv'] = ext("bv", [1, C])
    d['ok'] = ext("ok", [C, C])                    # out_kernel (hd, o)
    d['mask'] = ext("mask", [128, 128])            # identity mask for diag build
    d['ident'] = ext("ident", [112, 112])          # transpose identity
    out_d = nc.dram_tensor("out", [NTOK, C], F32, kind="ExternalOutput").ap()

    with nc.allow_low_precision(reason="bf16/fp8 rounding is intentional"), \
         tile.TileContext(nc) as tc, ExitStack() as ctx:
        wp = ctx.enter_context(tc.tile_pool(name="wp", bufs=1))
        stg = ctx.enter_context(tc.tile_pool(name="stg", bufs=2))
        xp = ctx.enter_context(tc.tile_pool(name="xp", bufs=3))
        padp = ctx.enter_context(tc.tile_pool(name="padp", bufs=1))
        dgp = ctx.enter_context(tc.tile_pool(name="dgp", bufs=1))
        dwo = ctx.enter_context(tc.tile_pool(name="dwo", bufs=1))
        dwq = ctx.enter_context(tc.tile_pool(name="dwq", bufs=2))
        actp = ctx.enter_context(tc.tile_pool(name="actp", bufs=1))
        qtp = ctx.enter_context(tc.tile_pool(name="qtp", bufs=2))
        ptp = ctx.enter_context(tc.tile_pool(name="ptp", bufs=1))
        smp = ctx.enter_context(tc.tile_pool(name="smp", bufs=2))
        ap_ = ctx.enter_context(tc.tile_pool(name="ap", bufs=2))
        outp = ctx.enter_context(tc.tile_pool(name="outp", bufs=2))
        psu = ctx.enter_context(tc.tile_pool(name="psu", bufs=1, space="PSUM"))
        _n = [0]

        def nm(s):
            _n[0] += 1
            return f"{s}_{_n[0]}"

        def load_f32(name, shape, tag):
            t = wp.tile(shape, F32, tag=tag, name=nm(tag))
            nc.sync.dma_start(t[:], d[name][:, :])
            return t

        def load_cast(name, shape, tag, dt=BF16, rows=None):
            s = stg.tile(shape, F32, tag="stg", name=nm("stg"))
            src = d[name][:, :] if rows is None else d[name][rows[0]:rows[1], :]
            nc.sync.dma_start(s[:], src)
            t = wp.tile(shape, dt, tag=tag, name=nm(tag))
            nc.vector.tensor_copy(t[:], s[:])
            return t

        ident = load_f32('ident', [112, 112], "ident")
        identb = wp.tile([112, 112], BF16, tag="identb", name=nm("identb"))
        nc.vector.tensor_copy(identb[:], ident[:])

        ones1 = wp.tile([1, KVC], F32, tag="ones1", name=nm("ones1"))
        nc.vector.memset(ones1[:], 1.0)
        ones1r = wp.tile([1, KVC], BF16, tag="ones1r", name=nm("ones1r"))
        nc.vector.tensor_copy(ones1r[:], ones1[:])
        onesv = wp.tile([112, NH], F32, tag="onesv", name=nm("onesv"))
        nc.vector.memset(onesv[:], 1.0)
        zrow = wp.tile([128, PW], F32, tag="zrow", name=nm("zrow"))
        nc.vector.memset(zrow[:], 0.0)

        def zero_border(pad):
            pv = pad[:].rearrange("p (r c) -> p r c", c=PW)
            zr = zrow[:].rearrange("p (a c) -> p a c", a=1)
            zc = zrow[:].rearrange("p (c a) -> p c a", a=1)
            nc.vector.tensor_copy(pv[:, 0:1, :], zr)
            nc.vector.tensor_copy(pv[:, PW - 1:PW, :], zr)
            nc.vector.tensor_copy(pv[:, :, 0:1], zc)
            nc.vector.tensor_copy(pv[:, :, PW - 1:PW], zc)

        def transpose_in(src_dram, pads, scatter_engines):
            """DMA token-major rows, cast bf16, PE-transpose, scatter into pads."""
            for g in range(28):  # 112 tokens = 2 image rows per group
                xs = xp.tile([112, C], F32, tag="xs", name=nm("xs"))
                nc.sync.dma_start(xs[:], src_dram[g * 112:(g + 1) * 112, :])
                xb = xp.tile([112, C], BF16, tag="xb", name=nm("xb"))
                if g % 2 == 0:
                    nc.vector.tensor_copy(xb[:], xs[:])
                else:
                    nc.scalar.copy(xb[:], xs[:])
                for ch in range(3):
                    pt = psu.tile([128, 112], BF16, tag="qk", name=nm("qk"), bufs=2)
                    nc.tensor.transpose(pt[:], xb[:, ch * 128:(ch + 1) * 128], identb[:])
                    pv = pads[ch][:].rearrange("p (r c) -> p r c", c=PW)
                    dst = pv[:, 2 * g + 1:2 * g + 3, 1:57]
                    src = pt[:].rearrange("p (r c) -> p r c", c=56)
                    eng = scatter_engines[(g + ch) % len(scatter_engines)]
                    if eng == 'v':
                        nc.vector.tensor_copy(dst, src)
                    else:
                        nc.scalar.copy(dst, src)

        def build_diags(p):
            tg = 'q' if p == 'q' else 'kv'
            dgs = []
            for ch in range(3):
                row = []
                for tap in range(9):
                    dg = dgp.tile([128, 128], BF16, tag=f"dg{tg}{ch}_{tap}",
                                  name=nm(f"dg{p}{ch}_{tap}"))
                    nc.vector.tensor_scalar(
                        dg[:], mask[:], dwv[p][:, tap * 3 + ch:tap * 3 + ch + 1],
                        None, op0=MUL)
                    row.append(dg)
                dgs.append(row)
            return dgs

        # ---- KV input feed first: x DMAs ahead of weight DMAs ----
        pads_kv = [padp.tile([128, PW * PW], BF16, tag=f"pad{ch}", name=nm(f"pad{ch}")) for ch in range(3)]
        for ch in range(3):
            zero_border(pads_kv[ch])
        transpose_in(d['xkv'], pads_kv, ['v', 's'])

        mask = load_f32('mask', [128, 128], "mask")
        dwv = {p: load_f32(f'dwv{p}', [128, 27], f"dwv{p}") for p in 'qkv'}
        bq = load_f32('bq', [128, 3], "bq")
        bk = load_f32('bk', [128, 3], "bk")
        bv = load_cast('bv', [1, C], "bv")
        wmat = {p: [load_cast(f'w{p}', [128, C], f"w{p}{c}", rows=(c * 128, (c + 1) * 128))
                    for c in range(3)] for p in 'qkv'}
        okm = [load_cast('ok', [128, C], f"ok{c}", rows=(c * 128, (c + 1) * 128))
               for c in range(3)]

        pads_q = [padp.tile([128, PW * PW], BF16, tag=f"padq{ch}", name=nm(f"padq{ch}")) for ch in range(3)]
        for ch in range(3):
            zero_border(pads_q[ch])
        transpose_in(d['xq'], pads_q, ['v', 's'])

        kvdw = {}
        for p in 'kv':
            dgs = build_diags(p)
            for ch in range(3):
                ot = dwo.tile([128, NKV], BF16, tag=f"kvdw_{p}{ch}", name=nm(f"kvdw_{p}{ch}"))
                kvdw[(p, ch)] = ot
                for half in range(2):  # 392 tokens = 14 out rows of 28
                    ps = psu.tile([128, 392], F32, tag="pb", name=nm("pb"), bufs=2)
                    pv = pads_kv[ch][:].rearrange("p (r c) -> p r c", c=PW)
                    for tap in range(9):
                        dy, dx = tap // 3, tap % 3
                        y0 = half * 14
                        rv = pv[:, 2 * y0 + dy + 1: 2 * y0 + dy + 28:2,
                                dx + 1: dx + 56:2]
                        nc.tensor.matmul(ps[:], dgs[ch][tap][:], rv,
                                         start=(tap == 0), stop=(tap == 8))
                    dst = ot[:, half * 392:(half + 1) * 392]
                    if (ch + half) % 2 == 0:
                        nc.vector.tensor_copy(dst, ps[:])
                    else:
                        nc.scalar.copy(dst, ps[:])

        # pw-k: channel-major kT [3][128, 784], bias added on DVE
        kT = []
        for co in range(3):
            kt = actp.tile([128, NKV], BF16, tag=f"kT{co}", name=nm(f"kT{co}"))
            kT.append(kt)
            for half in range(2):
                ps = psu.tile([128, 392], F32, tag="pb", name=nm("pb"), bufs=2)
                for ci in range(3):
                    nc.tensor.matmul(
                        ps[:], wmat['k'][ci][:, co * 128:(co + 1) * 128],
                        kvdw[('k', ci)][:, half * 392:(half + 1) * 392],
                        start=(ci == 0), stop=(ci == 2))
                nc.vector.tensor_scalar(kt[:, half * 392:(half + 1) * 392], ps[:],
                                        bk[:, co:co + 1], None, op0=ADD)

        # pw-v: token-major v' [7][112, 6*65] with ones col per head
        vs = []
        for j in range(NKVC):
            ps = psu.tile([112, C], F32, tag="pb", name=nm("pb"), bufs=2)
            for ci in range(3):
                nc.tensor.matmul(ps[:], kvdw[('v', ci)][:, j * KVC:(j + 1) * KVC],
                                 wmat['v'][ci][:], start=(ci == 0), stop=False)
            nc.tensor.matmul(ps[:], ones1r[:], bv[:], start=False, stop=True)
            vt = actp.tile([112, NH * 65], BF16, tag=f"vs{j}", name=nm(f"vs{j}"))
            vs.append(vt)
            vv = vt[:].rearrange("p (h e) -> p h e", e=65)
            nc.scalar.copy(vv[:, :, 0:64],
                           ps[:].rearrange("p (h e) -> p h e", e=64))
            nc.scalar.copy(vv[:, :, 64:65],
                           onesv[:].rearrange("p (h e) -> p h e", e=1))

        # ---- Q path + attention + projection, per 448-token tile ----
        # Software-pipelined: dw-q matmuls of tile t+1 are interleaved between
        # the attention channel groups of tile t so the PE has fill work while
        # the scalar engine drains the exp queue; pw-q of t+1 follows op(t).
        dgs_q = build_diags('q')

        def do_dwq_ch(t, ch):
            # taps 0-6 accumulate on the PE; taps 7-8 run on the DVE as fused
            # per-partition multiply-adds, merged during the PSUM evacuation.
            y0 = t * 8
            ps = psu.tile([128, QT], F32, tag="pb", name=nm("pb"), bufs=2)
            pv = pads_q[ch][:].rearrange("p (r c) -> p r c", c=PW)
            for tap in range(7):
                dy, dx = tap // 3, tap % 3
                rv = pv[:, y0 + dy:y0 + dy + 8, dx:dx + 56]
                nc.tensor.matmul(ps[:], dgs_q[ch][tap][:], rv,
                                 start=(tap == 0), stop=(tap == 6))
            part = dwq.tile([128, QT], F32, tag=f"dvp{ch}", name=nm(f"dvp{ch}"), bufs=2)
            rv7 = pv[:, y0 + 2:y0 + 2 + 8, 1:57]
            nc.vector.tensor_scalar(part[:], rv7, dwv['q'][:, 7 * 3 + ch:7 * 3 + ch + 1],
                                    None, op0=MUL)
            rv8 = pv[:, y0 + 2:y0 + 2 + 8, 2:58]
            nc.vector.scalar_tensor_tensor(part[:], rv8,
                                           dwv['q'][:, 8 * 3 + ch:8 * 3 + ch + 1],
                                           part[:], op0=MUL, op1=ADD)
            dt_ = dwq.tile([128, QT], BF16, tag=f"dwq{ch}", name=nm(f"dwq{ch}"), bufs=3)
            nc.vector.tensor_tensor(dt_[:], ps[:], part[:], op=ADD)
            return dt_

        def do_pwq(dq):
            qt_ = []
            for co in range(3):
                ps = psu.tile([128, QT], F32, tag="pb", name=nm("pb"), bufs=2)
                for ci in range(3):
                    nc.tensor.matmul(ps[:], wmat['q'][ci][:, co * 128:(co + 1) * 128],
                                     dq[ci][:], start=(ci == 0), stop=(ci == 2))
                qtt = qtp.tile([128, QT], BF16, tag=f"qt{co}", name=nm(f"qt{co}"), bufs=7)
                qt_.append(qtt)
                nc.scalar.activation(qtt[:], ps[:], AF.Identity, bias=bq[:, co:co + 1])
            return qt_

        qt_ = do_pwq([do_dwq_ch(0, ch) for ch in range(3)])

        for t in range(NQT):
            dq_next = [None, None, None]
            # attention: heads processed in base-partition pairs; QK row-packed
            at_ = [ap_.tile([128, QT], BF16, tag=f"at{ch}", name=nm(f"at{ch}")) for ch in range(3)]
            for ch in range(3):
                hpts = {0: [], 1: []}
                for j2 in range(3):
                    pss = {}
                    for half in range(2):
                        lo = half * 64
                        ps = psu.tile([112, 1024], F32, tag="qk", name=nm("qk"), bufs=2)
                        pss[half] = ps
                        for k in range(2):
                            j = 2 * j2 + k
                            nc.tensor.matmul(ps[:, k * 512:k * 512 + QT],
                                             kT[ch][lo:lo + 64, j * KVC:(j + 1) * KVC],
                                             qt_[ch][lo:lo + 64, :], start=True, stop=True,
                                             skip_group_check=True)
                    for half in range(2):
                        pt = ptp.tile([112, 2 * QT], BF16, tag=f"pt{half}_{j2}",
                                      name=nm(f"pt{half}_{j2}"), bufs=2)
                        hpts[half].append(pt)
                        nc.scalar.activation(
                            pt[:].rearrange("p (a q) -> p a q", a=2),
                            pss[half][:].rearrange("p (a q) -> p a q", a=2)[:, :, 0:QT],
                            AF.Exp)
                pt6s = {}
                for half in range(2):
                    lo = half * 64
                    ps = psu.tile([112, 1024], F32, tag="qk", name=nm("qk"), bufs=2)
                    nc.tensor.matmul(ps[:, 0:QT], kT[ch][lo:lo + 64, 6 * KVC:7 * KVC],
                                     qt_[ch][lo:lo + 64, :], start=True, stop=True)
                    pt6 = ptp.tile([112, QT], BF16, tag=f"pt{half}_3",
                                   name=nm(f"pt{half}_3"), bufs=2)
                    pt6s[half] = pt6
                    nc.scalar.activation(pt6[:], ps[:, 0:QT], AF.Exp)
                avs, bcs = {}, {}
                for half in range(2):
                    h = 2 * ch + half
                    pts = hpts[half]
                    pt6 = pt6s[half]
                    av = psu.tile([65, QT], F32, tag="mp", name=nm("mp"), bufs=2)
                    avs[half] = av
                    for j in range(NKVC):
                        rhs = pt6[:] if j == 6 else pts[j // 2][:, (j % 2) * QT:(j % 2) * QT + QT]
                        nc.tensor.matmul(av[:], vs[j][:, h * 65:(h + 1) * 65], rhs,
                                         start=(j == 0), stop=(j == NKVC - 1))
                    den = smp.tile([1, QT], F32, tag="den", name=nm("den"))
                    nc.vector.tensor_copy(den[:], av[64:65, :])
                    rec = smp.tile([1, QT], F32, tag="rec", name=nm("rec"))
                    nc.vector.reciprocal_approx_fast(rec[:], den[:])
                    bc = smp.tile([64, QT], F32, tag="bc", name=nm("bc"), bufs=2)
                    bcs[half] = bc
                    nc.gpsimd.partition_broadcast(bc[:], rec[:])
                # dw-q of the next tile: PE fill work during exp/broadcast; its
                # vector evac is issued before the broadcast-gated TT mults so
                # the pb PSUM bank frees without waiting on gpsimd.
                if t + 1 < NQT:
                    dq_next[ch] = do_dwq_ch(t + 1, ch)
                for half in range(2):
                    nc.vector.tensor_tensor(at_[ch][half * 64:half * 64 + 64, :],
                                            avs[half][0:64, :], bcs[half][:],
                                            op=MUL)

            if t + 1 < NQT:
                qt_ = do_pwq(dq_next)
            # out projection, token-major (bf16 stationary + moving)
            for i, qn in ((0, 128), (1, 128), (2, 128), (3, 64)):
                ps = psu.tile([qn, C], F32, tag="mp", name=nm("mp"), bufs=2)
                for ch in range(3):
                    nc.tensor.matmul(ps[:], at_[ch][:, i * 128:i * 128 + qn],
                                     okm[ch][:], start=(ch == 0), stop=(ch == 2))
                ot = outp.tile([qn, C], F32, tag="ot", name=nm("ot"))
                nc.vector.tensor_copy(ot[:], ps[:])
                nc.sync.dma_start(out_d[t * QT + i * 128:t * QT + i * 128 + qn, :],
                                  ot[:])

    nc.compile()
    return nc


def _fold_weights(inputs):
    g = lambda n: np.asarray(inputs[n], dtype=np.float32)
    fold = {}
    for p in 'qkv':
        s = g(f'{p}_bn_scale') / np.sqrt(g(f'{p}_bn_var') + BN_EPS)
        t = g(f'{p}_bn_bias') - g(f'{p}_bn_mean') * s
        dw = g(f'{p}_dw_kernel').reshape(9, C) * s[None, :]         # (tap, c)
        wmat = g(f'{p}_pw_kernel').reshape(C, C)
        bias = t @ wmat
        if p == 'q':
            wmat = wmat / np.sqrt(np.float32(HD))
            bias = bias / np.sqrt(np.float32(HD))
        # dwv[part, tap*3+ch] = dw[tap, ch*128+part]
        dwv = dw.reshape(9, 3, 128).transpose(2, 0, 1).reshape(128, 27)
        fold[f'w{p}'] = np.ascontiguousarray(wmat)
        fold[f'dwv{p}'] = np.ascontiguousarray(dwv)
        fold[f'b{p}'] = bias
    common = {
        'wq': fold['wq'], 'wk': fold['wk'], 'wv': fold['wv'],
        'dwvq': fold['dwvq'], 'dwvk': fold['dwvk'], 'dwvv': fold['dwvv'],
        'bq': np.ascontiguousarray(fold['bq'].reshape(3, 128).T),
        'bk': np.ascontiguousarray(fold['bk'].reshape(3, 128).T),
        'bv': fold['bv'].reshape(1, C),
        'ok': np.ascontiguousarray(np.asarray(inputs['out_kernel'],
                                              dtype=np.float32).reshape(C, C)),
        'mask': np.eye(128, dtype=np.float32),
        'ident': np.eye(112, dtype=np.float32),
    }
    return common


def kernel(**inputs):
    if 'nc' not in _cache:
        _cache['nc'] = _build_nc()
    nc = _cache['nc']
    common = _fold_weights(inputs)
    xq = np.asarray(inputs['inputs_q'], dtype=np.float32).reshape(B, NTOK, C)
    xkv = np.asarray(inputs['inputs_kv'], dtype=np.float32).reshape(B, NTOK, C)
    in_maps = [dict(common, xq=np.ascontiguousarray(xq[b]),
                    xkv=np.ascontiguousarray(xkv[b])) for b in range(B)]
    res = run_bass_kernel_spmd(nc, in_maps, list(range(B)), trace=False)
    out = np.stack([res.results[b]['out'] for b in range(B)], axis=0)
    return out.astype(np.float32)
